# revision 2
# baseline (speedup 1.0000x reference)
"""MoE gate (DeepSeek-style grouped top-k routing) on 8 Trainium2 NeuronCores.

Strategy (per core, tokens sharded 8-way -> 2048 tokens/core):
  - X [2048, 4096] loaded naturally (tokens on partitions), PE-transposed in
    [128,128] blocks to build X^T tiles (hidden on partitions).
  - fp32 matmuls W^T-chunk-stationary / X^T-moving accumulate logits^T
    [256 experts, 512 tokens] in PSUM over 32 k-chunks.
  - logits^T copied to SBUF, PE-transposed back to [128 tokens, 256 experts].
  - ACT sigmoid -> scores; DVE chain does the grouped top-k exactly:
    group top-2 sums (reduce_max + match_replace + reduce_max), top-4 group
    mask via max8 + is_lt + copy_predicated(-10), expert top-8 via max8 +
    max_index, ordered weights via a second max8/max_index pass on the
    sigmoid-masked tile + compact 8x8 matching, normalize * 2.5.
  - Outputs stored tile-major [128, 16, 8]; host reorders to [2048, 8].
"""
import numpy as np

H = 4096            # hidden
E = 256             # experts
G = 8               # groups
EPG = 32            # experts per group
K = 8               # top-k
NCORES = 8
T_CORE = 2048       # tokens per core
BLK = 512           # tokens per matmul block
NBLK = T_CORE // BLK
NSUB = BLK // 128   # 128-token subtiles per block
NK = H // 128       # contraction chunks
SCALING = 2.5

_cache = {}


def _build_nc():
    import concourse.bacc as bacc
    import concourse.mybir as mybir
    from concourse.tile import TileContext

    f32 = mybir.dt.float32
    u16 = mybir.dt.uint16
    u32 = mybir.dt.uint32
    i32 = mybir.dt.int32
    AF = mybir.ActivationFunctionType
    ALU = mybir.AluOpType
    AX = mybir.AxisListType

    nc = bacc.Bacc(None)
    x_d = nc.declare_dram_parameter("x", [T_CORE, H], f32, isOutput=False)
    wt_d = nc.declare_dram_parameter("wt", [H, E], f32, isOutput=False)
    bias_d = nc.declare_dram_parameter("biasb", [128, E], f32, isOutput=False)
    id_d = nc.declare_dram_parameter("ident", [128, 128], f32, isOutput=False)
    oi_d = nc.declare_dram_parameter("out_idx", [128, T_CORE // 128, K], i32,
                                     isOutput=True)
    ow_d = nc.declare_dram_parameter("out_w", [128, T_CORE // 128, K], f32,
                                     isOutput=True)

    with TileContext(nc) as tc:
        with (
            tc.tile_pool(name="pers", bufs=1) as pers,
            tc.tile_pool(name="xnat", bufs=6) as xnat_p,
            tc.tile_pool(name="xt", bufs=3) as xt_p,
            tc.tile_pool(name="lt", bufs=4) as lt_p,
            tc.tile_pool(name="sc", bufs=4) as sc_p,
            tc.tile_pool(name="dve", bufs=2) as dve_p,
            tc.tile_pool(name="small", bufs=2) as small_p,
            tc.tile_pool(name="mmpsum", bufs=4, space="PSUM") as mm_ps,
            tc.tile_pool(name="tpsum", bufs=2, space="PSUM") as t_ps,
            tc.tile_pool(name="lpsum", bufs=2, space="PSUM") as l_ps,
        ):
            wt = pers.tile([128, NK, E], f32)
            nc.sync.dma_start(wt[:], wt_d[:].rearrange("(k p) e -> p k e", p=128))
            biasb = pers.tile([128, E], f32)
            nc.sync.dma_start(biasb[:], bias_d[:])
            ident = pers.tile([128, 128], f32)
            nc.sync.dma_start(ident[:], id_d[:])
            neg10 = pers.tile([128, 1], f32)
            nc.vector.memset(neg10[:], -10.0)

            idx_all = pers.tile([128, T_CORE // 128, K], i32)
            w_all = pers.tile([128, T_CORE // 128, K], f32)

            for b in range(NBLK):
                # ---- load X subtiles (natural layout) ----
                xs = []
                for s in range(NSUB):
                    xn = xnat_p.tile([128, H], f32, tag="xn")
                    r0 = b * BLK + s * 128
                    nc.sync.dma_start(xn[:], x_d[r0:r0 + 128, :])
                    xs.append(xn)

                # ---- matmul accumulators: logits^T [2][128e, BLK t] ----
                accs = [mm_ps.tile([128, BLK], f32, tag="acc", name=f"acc{b}_{i}") for i in range(2)]

                for k in range(NK):
                    # transpose X[:, k-chunk] -> XT_k [128k, BLK t]
                    tp = t_ps.tile([128, BLK], f32, tag="tp")
                    for s in range(NSUB):
                        nc.tensor.transpose(
                            tp[:, s * 128:(s + 1) * 128],
                            xs[s][:, k * 128:(k + 1) * 128],
                            ident[:])
                    xt = xt_p.tile([128, BLK], f32, tag="xt")
                    nc.scalar.copy(xt[:], tp[:])
                    for e in range(2):
                        nc.tensor.matmul(
                            accs[e][:],
                            wt[:, k, e * 128:(e + 1) * 128],
                            xt[:],
                            start=(k == 0), stop=(k == NK - 1))

                # ---- logits^T -> SBUF -> transpose -> sigmoid + topk ----
                lts = []
                for e in range(2):
                    lt = lt_p.tile([128, BLK], f32, tag="lt")
                    nc.scalar.copy(lt[:], accs[e][:])
                    lts.append(lt)

                for s in range(NSUB):
                    lg = l_ps.tile([128, E], f32, tag="lg")
                    for e in range(2):
                        nc.tensor.transpose(
                            lg[:, e * 128:(e + 1) * 128],
                            lts[e][:, s * 128:(s + 1) * 128],
                            ident[:])
                    scores = sc_p.tile([128, E], f32, tag="scores")
                    nc.scalar.activation(scores[:], lg[:], AF.Sigmoid)

                    t = b * NSUB + s   # global 128-token tile id

                    sfc = dve_p.tile([128, E], f32, tag="sfc")
                    nc.vector.tensor_add(sfc[:], scores[:], biasb[:])

                    m1 = small_p.tile([128, G], f32, tag="m1")
                    nc.vector.tensor_reduce(
                        m1[:], sfc[:].rearrange("p (g e) -> p g e", g=G),
                        axis=AX.X, op=ALU.max, opt_input=False)
                    zap = dve_p.tile([128, E], f32, tag="zap")
                    nc.vector.match_replace(zap[:], m1[:], sfc[:], -10.0)
                    m2 = small_p.tile([128, G], f32, tag="m2")
                    nc.vector.tensor_reduce(
                        m2[:], zap[:].rearrange("p (g e) -> p g e", g=G),
                        axis=AX.X, op=ALU.max, opt_input=False)
                    gs = small_p.tile([128, G], f32, tag="gs")
                    nc.vector.tensor_add(gs[:], m1[:], m2[:])

                    srt8 = small_p.tile([128, 8], f32, tag="srt8")
                    nc.vector.max(srt8[:], gs[:])
                    inv = small_p.tile([128, G], u32, tag="inv")
                    nc.vector.tensor_scalar(inv[:], gs[:], srt8[:, 3:4], None,
                                            op0=ALU.is_lt)

                    tmp = dve_p.tile([128, E], f32, tag="tmp")
                    nc.vector.tensor_copy(tmp[:], sfc[:])
                    nc.vector.copy_predicated(
                        tmp[:].rearrange("p (g e) -> p g e", g=G),
                        inv[:].unsqueeze(2).to_broadcast([128, G, EPG]),
                        neg10[:].unsqueeze(2).to_broadcast([128, G, EPG]))

                    v8 = small_p.tile([128, K], f32, tag="v8")
                    nc.vector.max(v8[:], tmp[:])
                    idx16 = small_p.tile([128, K], u16, tag="idx16")
                    nc.vector.max_index(idx16[:], v8[:], tmp[:])
                    idxf = small_p.tile([128, K], f32, tag="idxf")
                    nc.vector.tensor_copy(idxf[:], idx16[:])

                    zap2 = dve_p.tile([128, E], f32, tag="zap2")
                    nc.vector.match_replace(zap2[:], v8[:], tmp[:], -10.0)
                    posm = dve_p.tile([128, E], f32, tag="posm")
                    nc.vector.tensor_sub(posm[:], tmp[:], zap2[:])
                    mask01 = dve_p.tile([128, E], f32, tag="mask01")
                    nc.vector.tensor_scalar(mask01[:], posm[:], 0.0, None,
                                            op0=ALU.is_gt)
                    sm = dve_p.tile([128, E], f32, tag="sm")
                    nc.vector.tensor_mul(sm[:], scores[:], mask01[:])
                    us8 = small_p.tile([128, K], f32, tag="us8")
                    nc.vector.max(us8[:], sm[:])
                    pos16 = small_p.tile([128, K], u16, tag="pos16")
                    nc.vector.max_index(pos16[:], us8[:], sm[:])
                    posf = small_p.tile([128, K], f32, tag="posf")
                    nc.vector.tensor_copy(posf[:], pos16[:])

                    w8 = small_p.tile([128, K], f32, tag="w8")
                    e8 = small_p.tile([128, K], f32, tag="e8")
                    t8 = small_p.tile([128, K], f32, tag="t8")
                    for j in range(K):
                        nc.vector.tensor_scalar(e8[:], posf[:], idxf[:, j:j + 1],
                                                None, op0=ALU.is_equal)
                        nc.vector.tensor_mul(t8[:], e8[:], us8[:])
                        nc.vector.tensor_reduce(w8[:, j:j + 1], t8[:],
                                                axis=AX.X, op=ALU.add)

                    den = small_p.tile([128, 1], f32, tag="den")
                    nc.vector.tensor_reduce(den[:], w8[:], axis=AX.X, op=ALU.add)
                    rec = small_p.tile([128, 1], f32, tag="rec")
                    nc.vector.reciprocal(rec[:], den[:])
                    nc.vector.tensor_scalar(w_all[:, t, :], w8[:], rec[:],
                                            SCALING, op0=ALU.mult, op1=ALU.mult)
                    nc.vector.tensor_copy(idx_all[:, t, :], idx16[:])

            nc.sync.dma_start(oi_d[:], idx_all[:])
            nc.sync.dma_start(ow_d[:], w_all[:])

    nc.finalize()
    return nc


def _get_runner():
    """Build (once) a cached jitted SPMD runner over 8 cores."""
    if "runner" in _cache:
        return _cache["runner"]

    import jax
    import jax.numpy as jnp
    from jax.sharding import Mesh, PartitionSpec
    from jax.experimental.shard_map import shard_map
    import concourse.mybir as mybir
    from concourse import bass2jax
    from concourse.bass2jax import _bass_exec_p, partition_id_tensor

    bass2jax.install_neuronx_cc_hook()
    nc = _build_nc()

    in_names = []
    out_names = []
    out_avals = []
    zero_out_shapes = []
    partition_name = nc.partition_id_tensor.name if nc.partition_id_tensor else None
    for alloc in nc.m.functions[0].allocations:
        if not isinstance(alloc, mybir.MemoryLocationSet):
            continue
        name = alloc.memorylocations[0].name
        if alloc.kind == "ExternalInput":
            if name != partition_name:
                in_names.append(name)
        elif alloc.kind == "ExternalOutput":
            out_names.append(name)
            out_avals.append(jax.core.ShapedArray(
                tuple(alloc.tensor_shape), mybir.dt.np(alloc.dtype)))
            zero_out_shapes.append(
                (tuple(alloc.tensor_shape), mybir.dt.np(alloc.dtype)))

    n_params = len(in_names)
    all_in_names = list(in_names) + list(out_names)
    if partition_name is not None:
        all_in_names.append(partition_name)

    def _body(*args):
        operands = list(args)
        if partition_name is not None:
            operands.append(partition_id_tensor())
        outs = _bass_exec_p.bind(
            *operands,
            out_avals=tuple(out_avals),
            in_names=tuple(all_in_names),
            out_names=tuple(out_names),
            lowering_input_output_aliases=(),
            sim_require_finite=True,
            sim_require_nnan=True,
            nc=nc,
        )
        return tuple(outs)

    devices = jax.devices()[:NCORES]
    mesh = Mesh(np.asarray(devices), ("core",))
    n_outs = len(out_names)
    in_specs = (PartitionSpec("core"),) * (n_params + n_outs)
    out_specs = (PartitionSpec("core"),) * n_outs
    donate = tuple(range(n_params, n_params + n_outs))
    jitted = jax.jit(
        shard_map(_body, mesh=mesh, in_specs=in_specs, out_specs=out_specs,
                  check_rep=False),
        donate_argnums=donate, keep_unused=True)

    runner = {
        "jit": jitted,
        "in_names": in_names,
        "out_names": out_names,
        "zero_out_shapes": zero_out_shapes,
    }
    _cache["runner"] = runner
    return runner


def _prep_inputs(hidden_states, weight, e_score_correction_bias):
    """Per-core input dict -> concatenated global arrays (axis 0)."""
    x = np.ascontiguousarray(hidden_states, dtype=np.float32).reshape(-1, H)
    wt = np.ascontiguousarray(weight.astype(np.float32, copy=False).T)
    biasb = np.broadcast_to(
        e_score_correction_bias.astype(np.float32, copy=False), (128, E)).copy()
    ident = np.eye(128, dtype=np.float32)
    per_core = {
        "x": [x[c * T_CORE:(c + 1) * T_CORE] for c in range(NCORES)],
        "wt": [wt] * NCORES,
        "biasb": [biasb] * NCORES,
        "ident": [ident] * NCORES,
    }
    return per_core


def run_device(hidden_states, weight, e_score_correction_bias):
    runner = _get_runner()
    per_core = _prep_inputs(hidden_states, weight, e_score_correction_bias)
    concat_in = [np.concatenate(per_core[name], axis=0)
                 for name in runner["in_names"]]
    concat_zeros = [np.zeros((NCORES * shp[0], *shp[1:]), dt)
                    for shp, dt in runner["zero_out_shapes"]]
    outs = runner["jit"](*concat_in, *concat_zeros)
    res = {name: np.asarray(o) for name, o in zip(runner["out_names"], outs)}
    return res


def _assemble(res):
    NT = T_CORE // 128
    idx = res["out_idx"].reshape(NCORES, 128, NT, K)
    w = res["out_w"].reshape(NCORES, 128, NT, K)
    idx = idx.transpose(0, 2, 1, 3).reshape(NCORES * T_CORE, K)
    w = w.transpose(0, 2, 1, 3).reshape(NCORES * T_CORE, K)
    return idx.astype(np.int32), w.astype(np.float32)


def kernel(hidden_states, weight, e_score_correction_bias):
    res = run_device(np.asarray(hidden_states), np.asarray(weight),
                     np.asarray(e_score_correction_bias))
    return _assemble(res)


# revision 3
# speedup vs baseline: 737.2021x; 737.2021x over previous
"""MoE gate (DeepSeek-style grouped top-k routing) on 8 Trainium2 NeuronCores.

Strategy (per core, tokens sharded 8-way -> 2048 tokens/core):
  - X [2048, 4096] loaded naturally (tokens on partitions), PE-transposed in
    [128,128] blocks to build X^T tiles (hidden on partitions).
  - fp32 matmuls W^T-chunk-stationary / X^T-moving accumulate logits^T
    [256 experts, 512 tokens] in PSUM over 32 k-chunks.
  - logits^T copied to SBUF, PE-transposed back to [128 tokens, 256 experts].
  - ACT sigmoid -> scores; DVE chain does the grouped top-k exactly:
    group top-2 sums (reduce_max + match_replace + reduce_max), top-4 group
    mask via max8 + is_lt + copy_predicated(-10), expert top-8 via max8 +
    max_index, ordered weights via a second max8/max_index pass on the
    sigmoid-masked tile + compact 8x8 matching, normalize * 2.5.
  - Outputs stored tile-major [128, 16, 8]; host reorders to [2048, 8].
"""
import numpy as np

H = 4096            # hidden
E = 256             # experts
G = 8               # groups
EPG = 32            # experts per group
K = 8               # top-k
NCORES = 8
T_CORE = 2048       # tokens per core
BLK = 512           # tokens per matmul block
NBLK = T_CORE // BLK
NSUB = BLK // 128   # 128-token subtiles per block
NK = H // 128       # contraction chunks
SCALING = 2.5

_cache = {}


def _build_nc(reps=1):
    import concourse.bacc as bacc
    import concourse.mybir as mybir
    from concourse.tile import TileContext

    f32 = mybir.dt.float32
    u16 = mybir.dt.uint16
    u32 = mybir.dt.uint32
    i32 = mybir.dt.int32
    AF = mybir.ActivationFunctionType
    ALU = mybir.AluOpType
    AX = mybir.AxisListType

    nc = bacc.Bacc(None)
    x_d = nc.declare_dram_parameter("x", [T_CORE, H], f32, isOutput=False)
    wt_d = nc.declare_dram_parameter("wt", [H, E], f32, isOutput=False)
    bias_d = nc.declare_dram_parameter("biasb", [128, E], f32, isOutput=False)
    id_d = nc.declare_dram_parameter("ident", [128, 128], f32, isOutput=False)
    oi_d = nc.declare_dram_parameter("out_idx", [128, T_CORE // 128, K], i32,
                                     isOutput=True)
    ow_d = nc.declare_dram_parameter("out_w", [128, T_CORE // 128, K], f32,
                                     isOutput=True)

    import contextlib
    with TileContext(nc) as tc:
        rep_ctx = tc.For_i(0, reps, 1) if reps > 1 else contextlib.nullcontext()
        with (
            tc.tile_pool(name="pers", bufs=1) as pers,
            tc.tile_pool(name="xnat", bufs=6) as xnat_p,
            tc.tile_pool(name="xt", bufs=3) as xt_p,
            tc.tile_pool(name="lt", bufs=4) as lt_p,
            tc.tile_pool(name="sc", bufs=4) as sc_p,
            tc.tile_pool(name="dve", bufs=2) as dve_p,
            tc.tile_pool(name="small", bufs=2) as small_p,
            tc.tile_pool(name="mmpsum", bufs=4, space="PSUM") as mm_ps,
            tc.tile_pool(name="tpsum", bufs=2, space="PSUM") as t_ps,
            tc.tile_pool(name="lpsum", bufs=2, space="PSUM") as l_ps,
            rep_ctx,
        ):
            wt = pers.tile([128, NK, E], f32)
            nc.sync.dma_start(wt[:], wt_d[:].rearrange("(k p) e -> p k e", p=128))
            biasb = pers.tile([128, E], f32)
            nc.sync.dma_start(biasb[:], bias_d[:])
            ident = pers.tile([128, 128], f32)
            nc.sync.dma_start(ident[:], id_d[:])
            neg10 = pers.tile([128, 1], f32)
            nc.vector.memset(neg10[:], -10.0)

            idx_all = pers.tile([128, T_CORE // 128, K], i32)
            w_all = pers.tile([128, T_CORE // 128, K], f32)

            for b in range(NBLK):
                # ---- load X subtiles (natural layout) ----
                xs = []
                for s in range(NSUB):
                    xn = xnat_p.tile([128, H], f32, tag="xn")
                    r0 = b * BLK + s * 128
                    nc.sync.dma_start(xn[:], x_d[r0:r0 + 128, :])
                    xs.append(xn)

                # ---- matmul accumulators: logits^T [2][128e, BLK t] ----
                accs = [mm_ps.tile([128, BLK], f32, tag="acc", name=f"acc{b}_{i}") for i in range(2)]

                for k in range(NK):
                    # transpose X[:, k-chunk] -> XT_k [128k, BLK t]
                    tp = t_ps.tile([128, BLK], f32, tag="tp")
                    for s in range(NSUB):
                        nc.tensor.transpose(
                            tp[:, s * 128:(s + 1) * 128],
                            xs[s][:, k * 128:(k + 1) * 128],
                            ident[:])
                    xt = xt_p.tile([128, BLK], f32, tag="xt")
                    nc.scalar.copy(xt[:], tp[:])
                    for e in range(2):
                        nc.tensor.matmul(
                            accs[e][:],
                            wt[:, k, e * 128:(e + 1) * 128],
                            xt[:],
                            start=(k == 0), stop=(k == NK - 1))

                # ---- logits^T -> SBUF -> transpose -> sigmoid + topk ----
                lts = []
                for e in range(2):
                    lt = lt_p.tile([128, BLK], f32, tag="lt")
                    nc.scalar.copy(lt[:], accs[e][:])
                    lts.append(lt)

                for s in range(NSUB):
                    lg = l_ps.tile([128, E], f32, tag="lg")
                    for e in range(2):
                        nc.tensor.transpose(
                            lg[:, e * 128:(e + 1) * 128],
                            lts[e][:, s * 128:(s + 1) * 128],
                            ident[:])
                    scores = sc_p.tile([128, E], f32, tag="scores")
                    nc.scalar.activation(scores[:], lg[:], AF.Sigmoid)

                    t = b * NSUB + s   # global 128-token tile id

                    sfc = dve_p.tile([128, E], f32, tag="sfc")
                    nc.vector.tensor_add(sfc[:], scores[:], biasb[:])

                    m1 = small_p.tile([128, G], f32, tag="m1")
                    nc.vector.tensor_reduce(
                        m1[:], sfc[:].rearrange("p (g e) -> p g e", g=G),
                        axis=AX.X, op=ALU.max, opt_input=False)
                    zap = dve_p.tile([128, E], f32, tag="zap")
                    nc.vector.match_replace(zap[:], m1[:], sfc[:], -10.0)
                    m2 = small_p.tile([128, G], f32, tag="m2")
                    nc.vector.tensor_reduce(
                        m2[:], zap[:].rearrange("p (g e) -> p g e", g=G),
                        axis=AX.X, op=ALU.max, opt_input=False)
                    gs = small_p.tile([128, G], f32, tag="gs")
                    nc.vector.tensor_add(gs[:], m1[:], m2[:])

                    srt8 = small_p.tile([128, 8], f32, tag="srt8")
                    nc.vector.max(srt8[:], gs[:])
                    inv = small_p.tile([128, G], u32, tag="inv")
                    nc.vector.tensor_scalar(inv[:], gs[:], srt8[:, 3:4], None,
                                            op0=ALU.is_lt)

                    tmp = dve_p.tile([128, E], f32, tag="tmp")
                    nc.vector.tensor_copy(tmp[:], sfc[:])
                    nc.vector.copy_predicated(
                        tmp[:].rearrange("p (g e) -> p g e", g=G),
                        inv[:].unsqueeze(2).to_broadcast([128, G, EPG]),
                        neg10[:].unsqueeze(2).to_broadcast([128, G, EPG]))

                    v8 = small_p.tile([128, K], f32, tag="v8")
                    nc.vector.max(v8[:], tmp[:])
                    idx16 = small_p.tile([128, K], u16, tag="idx16")
                    nc.vector.max_index(idx16[:], v8[:], tmp[:])
                    idxf = small_p.tile([128, K], f32, tag="idxf")
                    nc.vector.tensor_copy(idxf[:], idx16[:])

                    zap2 = dve_p.tile([128, E], f32, tag="zap2")
                    nc.vector.match_replace(zap2[:], v8[:], tmp[:], -10.0)
                    posm = dve_p.tile([128, E], f32, tag="posm")
                    nc.vector.tensor_sub(posm[:], tmp[:], zap2[:])
                    mask01 = dve_p.tile([128, E], f32, tag="mask01")
                    nc.vector.tensor_scalar(mask01[:], posm[:], 0.0, None,
                                            op0=ALU.is_gt)
                    sm = dve_p.tile([128, E], f32, tag="sm")
                    nc.vector.tensor_mul(sm[:], scores[:], mask01[:])
                    us8 = small_p.tile([128, K], f32, tag="us8")
                    nc.vector.max(us8[:], sm[:])
                    pos16 = small_p.tile([128, K], u16, tag="pos16")
                    nc.vector.max_index(pos16[:], us8[:], sm[:])
                    posf = small_p.tile([128, K], f32, tag="posf")
                    nc.vector.tensor_copy(posf[:], pos16[:])

                    w8 = small_p.tile([128, K], f32, tag="w8")
                    e8 = small_p.tile([128, K], f32, tag="e8")
                    t8 = small_p.tile([128, K], f32, tag="t8")
                    for j in range(K):
                        nc.vector.tensor_scalar(e8[:], posf[:], idxf[:, j:j + 1],
                                                None, op0=ALU.is_equal)
                        nc.vector.tensor_mul(t8[:], e8[:], us8[:])
                        nc.vector.tensor_reduce(w8[:, j:j + 1], t8[:],
                                                axis=AX.X, op=ALU.add)

                    den = small_p.tile([128, 1], f32, tag="den")
                    nc.vector.tensor_reduce(den[:], w8[:], axis=AX.X, op=ALU.add)
                    rec = small_p.tile([128, 1], f32, tag="rec")
                    nc.vector.reciprocal(rec[:], den[:])
                    nc.vector.tensor_scalar(w_all[:, t, :], w8[:], rec[:],
                                            SCALING, op0=ALU.mult, op1=ALU.mult)
                    nc.vector.tensor_copy(idx_all[:, t, :], idx16[:])

            nc.sync.dma_start(oi_d[:], idx_all[:])
            nc.sync.dma_start(ow_d[:], w_all[:])

    nc.finalize()
    return nc


def _get_runner(reps=1):
    """Build (once per reps) a cached jitted SPMD runner over 8 cores."""
    key = ("runner", reps)
    if key in _cache:
        return _cache[key]

    import jax
    import jax.numpy as jnp
    from jax.sharding import Mesh, PartitionSpec
    from jax.experimental.shard_map import shard_map
    import concourse.mybir as mybir
    from concourse import bass2jax
    from concourse.bass2jax import _bass_exec_p, partition_id_tensor

    bass2jax.install_neuronx_cc_hook()
    nc = _build_nc(reps)

    in_names = []
    out_names = []
    out_avals = []
    zero_out_shapes = []
    partition_name = nc.partition_id_tensor.name if nc.partition_id_tensor else None
    for alloc in nc.m.functions[0].allocations:
        if not isinstance(alloc, mybir.MemoryLocationSet):
            continue
        name = alloc.memorylocations[0].name
        if alloc.kind == "ExternalInput":
            if name != partition_name:
                in_names.append(name)
        elif alloc.kind == "ExternalOutput":
            out_names.append(name)
            out_avals.append(jax.core.ShapedArray(
                tuple(alloc.tensor_shape), mybir.dt.np(alloc.dtype)))
            zero_out_shapes.append(
                (tuple(alloc.tensor_shape), mybir.dt.np(alloc.dtype)))

    n_params = len(in_names)
    all_in_names = list(in_names) + list(out_names)
    if partition_name is not None:
        all_in_names.append(partition_name)

    def _body(*args):
        operands = list(args)
        if partition_name is not None:
            operands.append(partition_id_tensor())
        outs = _bass_exec_p.bind(
            *operands,
            out_avals=tuple(out_avals),
            in_names=tuple(all_in_names),
            out_names=tuple(out_names),
            lowering_input_output_aliases=(),
            sim_require_finite=True,
            sim_require_nnan=True,
            nc=nc,
        )
        return tuple(outs)

    devices = jax.devices()[:NCORES]
    mesh = Mesh(np.asarray(devices), ("core",))
    n_outs = len(out_names)
    in_specs = (PartitionSpec("core"),) * (n_params + n_outs)
    out_specs = (PartitionSpec("core"),) * n_outs
    donate = tuple(range(n_params, n_params + n_outs))
    jitted = jax.jit(
        shard_map(_body, mesh=mesh, in_specs=in_specs, out_specs=out_specs,
                  check_rep=False),
        donate_argnums=donate, keep_unused=True)

    runner = {
        "jit": jitted,
        "in_names": in_names,
        "out_names": out_names,
        "zero_out_shapes": zero_out_shapes,
    }
    _cache[key] = runner
    return runner


def _prep_inputs(hidden_states, weight, e_score_correction_bias):
    """Per-core input dict -> concatenated global arrays (axis 0)."""
    x = np.ascontiguousarray(hidden_states, dtype=np.float32).reshape(-1, H)
    wt = np.ascontiguousarray(weight.astype(np.float32, copy=False).T)
    biasb = np.broadcast_to(
        e_score_correction_bias.astype(np.float32, copy=False), (128, E)).copy()
    ident = np.eye(128, dtype=np.float32)
    per_core = {
        "x": [x[c * T_CORE:(c + 1) * T_CORE] for c in range(NCORES)],
        "wt": [wt] * NCORES,
        "biasb": [biasb] * NCORES,
        "ident": [ident] * NCORES,
    }
    return per_core


def run_device(hidden_states, weight, e_score_correction_bias):
    runner = _get_runner()
    per_core = _prep_inputs(hidden_states, weight, e_score_correction_bias)
    concat_in = [np.concatenate(per_core[name], axis=0)
                 for name in runner["in_names"]]
    concat_zeros = [np.zeros((NCORES * shp[0], *shp[1:]), dt)
                    for shp, dt in runner["zero_out_shapes"]]
    outs = runner["jit"](*concat_in, *concat_zeros)
    res = {name: np.asarray(o) for name, o in zip(runner["out_names"], outs)}
    return res


def _assemble(res):
    NT = T_CORE // 128
    idx = res["out_idx"].reshape(NCORES, 128, NT, K)
    w = res["out_w"].reshape(NCORES, 128, NT, K)
    idx = idx.transpose(0, 2, 1, 3).reshape(NCORES * T_CORE, K)
    w = w.transpose(0, 2, 1, 3).reshape(NCORES * T_CORE, K)
    return idx.astype(np.int32), w.astype(np.float32)


def kernel(hidden_states, weight, e_score_correction_bias):
    res = run_device(np.asarray(hidden_states), np.asarray(weight),
                     np.asarray(e_score_correction_bias))
    return _assemble(res)
